# revision 1
# baseline (speedup 1.0000x reference)
"""Trainium2 Bass kernel for nn_LucaGPLMMultiheadAttention.

MHA with RoPE: S=2048, B=2, E=1024, H=16, hd=64, fp32 in/out.
Sharding: head-parallel across 8 cores (2 heads x 2 batch = 4 (b,h) pairs
per core). q/k/v projections column-split, out projection row-split with an
on-device ReduceScatter; host concatenates shards.

All on-device compute is fp16 (fp32 psum accumulate); the 2e-2 gate allows
it with ~12x margin. Structure is built around three hardware facts from
the cost model:
  - ACT exp() over the 16.8M score elements per core is an irreducible
    ~133us and paces the attention phase. PE emits scores one kt ahead of
    the av-accumulate so ACT never starves.
  - PE matmul cost = moving-operand columns. The av contraction uses the
    *probs tile as stationary* and v (plus a ones column -> row sums) as
    the 65-wide moving operand: half the PE time of the v-stationary
    orientation. attn comes out qi-natural; row-sum reciprocals are then
    per-PARTITION scalars (one cheap tensor_scalar per 128-qi tile) and a
    16x [128,64] PE transpose pass rebuilds attnT for the out-projection.
  - The XBAR DMA-transpose engine (16x128 tiles, 14ns/tile) ingests the
    query pre-transposed from DRAM: no PE transposes, no PSUM copy
    traffic, fp16 halves the bytes.
Engine balance: batch-0 projection evictions ride the idle ACT engine in
phase 1; batch-1 projections + rope are deferred thunks drained one per kt
into batch-0's attention stream (filling PE/DVE slack under the ACT
pacer), as are each block's out-projection matmuls. v's bias is folded
into bo on the host (exact: softmax weights sum to 1), and bo itself is
added host-side after the ReduceScatter.
"""

import os
import sys

sys.path.insert(0, "/opt/trn_rl_repo")

import numpy as np

S = 2048
B = 2
E = 1024
H = 16
HD = 64
NCORES = 8
HPC = H // NCORES  # heads per core = 2
EL = HPC * HD  # local embed slice = 128
SB = S * B  # 4096 rows
SHARD = S // NCORES  # 256 rows per (core, batch) after reduce-scatter
QB = 1024  # qi block size
NKT = S // 128  # 16 kj tiles per batch

_CACHE: dict = {}
LAST_RESULT = None


def _build_program(with_cc: bool = True):
    import concourse.mybir as mybir
    import concourse.tile as tile
    from concourse import bacc
    from concourse.masks import make_identity

    f32 = mybir.dt.float32
    f16 = mybir.dt.float16
    Exp = mybir.ActivationFunctionType.Exp
    Copy = mybir.ActivationFunctionType.Copy
    Ident = mybir.ActivationFunctionType.Identity
    add = mybir.AluOpType.add
    mult = mybir.AluOpType.mult

    nc = bacc.Bacc(
        "TRN2",
        target_bir_lowering=False,
        debug=False,
        enable_asserts=False,
        num_devices=NCORES,
    )

    def din(name, shape, dt=f16):
        return nc.dram_tensor(name, shape, dt, kind="ExternalInput").ap()

    query = din("query", [S, B, E])  # fp16 from host
    q_w = din("q_w", [E, EL])  # (Wq_slice * scaling).T
    k_w = din("k_w", [E, EL])
    v_w = din("v_w", [E, EL])
    o_w = din("o_w", [EL, E])  # Wo[:, slice].T
    bq_s = din("bq_s", [EL, 1], f32)
    bk_s = din("bk_s", [EL, 1], f32)
    cos_t = din("cos_t", [EL, S])  # 2-head stacked rope tables (sin sign-folded)
    sin_t = din("sin_t", [EL, S])
    out_ext = nc.dram_tensor("out", [B, SHARD, E], f16, kind="ExternalOutput").ap()

    with tile.TileContext(nc) as tc:
        with (
            tc.tile_pool(name="const", bufs=1) as const,
            tc.tile_pool(name="persist", bufs=1) as persist,
            tc.tile_pool(name="dram", bufs=1, space="DRAM") as dram,
        ):
            qw_sb = const.tile([128, 8, EL], f16, name="qw_sb")
            kw_sb = const.tile([128, 8, EL], f16, name="kw_sb")
            vw_sb = const.tile([128, 8, EL], f16, name="vw_sb")
            ow_sb = const.tile([EL, E], f16, name="ow_sb")
            bq_sb = const.tile([EL, 1], f32, name="bq_sb")
            bk_sb = const.tile([EL, 1], f32, name="bk_sb")
            cos_sb = const.tile([EL, S], f16, name="cos_sb")
            sin_sb = const.tile([EL, S], f16, name="sin_sb")
            id16 = const.tile([128, 128], f16, name="id16")

            # persistent activations
            qts = persist.tile([128, 8, SB], f16, name="qts")  # queryT
            qT = persist.tile([EL, SB], f16, name="qT")
            kT = persist.tile([EL, SB], f16, name="kT")
            vaug = persist.tile([128, B, NKT, HPC, HD + 1], f16, name="vaug")
            attnT = [
                [persist.tile([EL, QB], f16, name=f"attnT{b}_{qb}") for qb in range(2)]
                for b in range(B)
            ]
            P_dram = [dram.tile([S, E], f16, name=f"P_dram{b}") for b in range(B)]
            rs_out = [dram.tile([SHARD, E], f16, name=f"rs_out{b}") for b in range(B)]

            # DMA emission order: tiny tensors first, then weights, then the
            # b0 query-transposes at s-block granularity (the first projection
            # only needs 8 of them), then b1.
            def emit_qts(b, r0, rows):
                for ec in range(8):
                    nc.sync.dma_start_transpose(
                        qts[:, ec, b * S + r0 : b * S + r0 + rows],
                        query[r0 : r0 + rows, b, ec * 128 : (ec + 1) * 128],
                    )

            nc.sync.dma_start(bq_sb[:], bq_s[:])
            nc.sync.dma_start(bk_sb[:], bk_s[:])
            nc.sync.dma_start(cos_sb[:], cos_t[:])
            nc.sync.dma_start(sin_sb[:], sin_t[:])
            for src, dst in ((q_w, qw_sb), (k_w, kw_sb), (v_w, vw_sb)):
                nc.sync.dma_start(dst[:], src.rearrange("(c p) m -> p c m", p=128))
            for sblk in range(4):
                emit_qts(0, sblk * 512, 512)
            emit_qts(1, 0, 1024)
            emit_qts(1, 1024, 1024)
            nc.sync.dma_start(ow_sb[:], o_w[:])
            make_identity(nc, id16[:])
            nc.vector.memset(vaug[:, :, :, :, HD], 1.0)

            pending: list = []  # deferred emit thunks (b1 proj, out-proj)

            def emit_some(n):
                for _ in range(min(n, len(pending))):
                    pending.pop(0)()

            mix_ps = None  # assigned when the phase-2 PSUM pools open
            with (
                tc.tile_pool(name="probs", bufs=2) as probs_pool,
                tc.tile_pool(name="osb", bufs=3) as osb,
                tc.tile_pool(name="rope", bufs=2) as rope,
                tc.tile_pool(name="anat", bufs=4) as anat_pool,
                tc.tile_pool(name="nrm", bufs=2) as nrm,
            ):

                def do_rope(dst, cs, ccol, eng):
                    """x' = x*cos + shuffle(x)*sin_f on `eng` (DVE)."""
                    dcol = dst[:, cs]
                    shuf = rope.tile([EL, 512], f16, tag="shuf")
                    t1 = rope.tile([EL, 512], f16, tag="t1")
                    for h in range(HPC):
                        p0 = h * HD
                        eng.tensor_copy(
                            shuf[p0 : p0 + 32, :], dcol[p0 + 32 : p0 + 64, :]
                        )
                        eng.tensor_copy(
                            shuf[p0 + 32 : p0 + 64, :], dcol[p0 : p0 + 32, :]
                        )
                    eng.tensor_tensor(
                        out=t1[:], in0=dcol, in1=cos_sb[:, ccol], op=mult
                    )
                    eng.tensor_tensor(
                        out=shuf[:], in0=shuf[:], in1=sin_sb[:, ccol], op=mult
                    )
                    eng.tensor_tensor(out=dcol, in0=t1[:], in1=shuf[:], op=add)

                def proj_qk(b, sblk, w_sb, bias, dst, act_evict, ps_pool):
                    col0 = b * S + sblk * 512
                    cs = slice(col0, col0 + 512)
                    ps = (
                        mix_tile()
                        if ps_pool is mix_ps
                        else ps_pool.tile([128, 512], f32, tag="pj")
                    )
                    for ec in range(8):
                        nc.tensor.matmul(
                            ps[:],
                            w_sb[:, ec, :],
                            qts[:, ec, cs],
                            start=(ec == 0),
                            stop=(ec == 7),
                        )
                    if act_evict:  # ACT idle during phase 1
                        nc.scalar.activation(dst[:, cs], ps[:], Ident, bias=bias[:])
                    else:  # thunked into phase 2: ACT is the pacer there
                        nc.vector.tensor_scalar_add(dst[:, cs], ps[:], bias[:])
                    do_rope(dst, cs, slice(sblk * 512, (sblk + 1) * 512), nc.vector)

                def mix_tile():
                    # mix_ps serves pj/vt/op/tr shapes from ONE tag so the
                    # pool stays at 2 banks; callers slice/bitcast the view
                    assert mix_ps is not None
                    return mix_ps.tile([128, 512], f32, tag="mix", name="mix")

                def proj_v(b, sblk, act_evict, ps_pool):
                    # v natural (s on partitions): stationary/moving swapped
                    for sc2 in range(4):
                        kt = sblk * 4 + sc2
                        c0 = b * S + sblk * 512 + sc2 * 128
                        vt = (
                            mix_tile()[:, 0:128]
                            if ps_pool is mix_ps
                            else ps_pool.tile([128, 128], f32, tag="vt")
                        )
                        for ec in range(8):
                            nc.tensor.matmul(
                                vt[:],
                                qts[:, ec, c0 : c0 + 128],
                                vw_sb[:, ec, :],
                                start=(ec == 0),
                                stop=(ec == 7),
                            )
                        dst = vaug[:, b, kt, :, 0:HD]
                        src = vt[:].rearrange("p (h d) -> p h d", h=HPC)
                        if act_evict:
                            nc.scalar.activation(dst, src, Copy)
                        else:
                            nc.vector.tensor_copy(dst, src)

                # ---- phase 1: ONLY the q/k projections the first exps need
                # (b0 s-blocks 0/1, evictions on the still-idle ACT). All
                # other projections become thunks, ordered so each tensor
                # lands just before its first reader in the attention stream.
                with tc.tile_pool(name="pj_ps", bufs=3, space="PSUM") as pj_ps:
                    for sblk in range(4):
                        # only sblk0/1 gate the first exps; their evicts ride
                        # ACT, the rest go to DVE to stay out of exp's way
                        act_ev = sblk < 2
                        proj_qk(0, sblk, qw_sb, bq_sb, qT, act_ev, pj_ps)
                        proj_qk(0, sblk, kw_sb, bk_sb, kT, act_ev, pj_ps)

                def mk_pj(b, sblk, w_sb, bias, dst):
                    return lambda: proj_qk(b, sblk, w_sb, bias, dst, False, mix_ps)

                def mk_v(b, sblk, sc2):
                    def thunk():
                        kt = sblk * 4 + sc2
                        c0 = b * S + sblk * 512 + sc2 * 128
                        vt = mix_tile()[:, 0:128]
                        for ec in range(8):
                            nc.tensor.matmul(
                                vt[:],
                                qts[:, ec, c0 : c0 + 128],
                                vw_sb[:, ec, :],
                                start=(ec == 0),
                                stop=(ec == 7),
                            )
                        nc.vector.tensor_copy(
                            vaug[:, b, kt, :, 0:HD],
                            vt[:].rearrange("p (h d) -> p h d", h=HPC),
                        )

                    return thunk

                # b0 sblk2/3 q/k first (kT cols needed from kt=8 of unit 0),
                # then b0 v (needed by unit 1's av groups), then all of b1
                # (needed from unit 4).
                for sblk in range(4):
                    for sc2 in range(4):
                        pending.append(mk_v(0, sblk, sc2))
                for sblk in range(4):
                    pending.append(mk_pj(1, sblk, qw_sb, bq_sb, qT))
                    pending.append(mk_pj(1, sblk, kw_sb, bk_sb, kT))
                    for sc2 in range(4):
                        pending.append(mk_v(1, sblk, sc2))

                def make_outproj(b, qb, tail=False):
                    def emit(st2):
                        def thunk():
                            stage = osb.tile([128, E], f16, tag="pstage")
                            for nch in range(2):
                                ps = mix_tile()
                                nc.tensor.matmul(
                                    ps[:],
                                    attnT[b][qb][:, st2 * 128 : (st2 + 1) * 128],
                                    ow_sb[:, nch * 512 : (nch + 1) * 512],
                                    start=True,
                                    stop=True,
                                    skip_group_check=True,
                                )
                                dst = stage[:, nch * 512 : (nch + 1) * 512]
                                # in the post-exp tail ACT is idle: share evicts
                                if tail and nch == 0:
                                    nc.scalar.activation(dst, ps[:], Copy)
                                else:
                                    nc.vector.tensor_copy(dst, ps[:])
                            st = qb * 8 + st2
                            nc.sync.dma_start(
                                P_dram[b][st * 128 : (st + 1) * 128, :], stage[:]
                            )

                        return thunk

                    return [emit(st2) for st2 in range(8)]

                def phase3(b):
                    if with_cc:
                        # collectives may not read/write IO tensors directly
                        nc.gpsimd.collective_compute(
                            "ReduceScatter",
                            add,
                            replica_groups=[list(range(NCORES))],
                            ins=[P_dram[b].opt()],
                            outs=[rs_out[b].opt()],
                        )
                        nc.sync.dma_start(out_ext[b], rs_out[b][:])
                    else:  # timeline-sim variant: no collective, copy shard 0
                        nc.sync.dma_start(out_ext[b], P_dram[b][0:SHARD, :])

                # ---- phase 2: attention, ACT-paced ----
                ph2 = tc.tile_pool(name="sc_ps", bufs=2, space="PSUM")
                sc_ps = ph2.__enter__()
                ph2b = tc.tile_pool(name="at_ps", bufs=2, space="PSUM")
                at_ps = ph2b.__enter__()
                ph2c = tc.tile_pool(name="mix_ps", bufs=2, space="PSUM")
                mix_ps = ph2c.__enter__()
                # HW rule (probed): an accumulation group's start=True zeroes
                # its whole PSUM BANK -> one open group per bank, evicted
                # before that bank's next group starts. Each head's 16 exp'd
                # prob tiles are therefore buffered whole, and the PREVIOUS
                # head's 8 qt-groups run serially (two at_ps banks ping-pong)
                # inside the current head's score/exp stream.
                units = [
                    (b, qb, h) for b in range(B) for qb in range(2)
                    for h in range(HPC)
                ]
                a_nats: dict = {}  # (b, qb) -> [a_nat_h0, a_nat_h1]

                def av_groups(b, qb, h, prb):
                    a_nat = anat_pool.tile(
                        [128, 8, HD], f16, tag="anat", name="a_nat"
                    )
                    a_nats.setdefault((b, qb), []).append(a_nat)

                    def grp(qt):
                        def thunk():
                            atq = at_ps.tile([128, 512], f32, tag="atq", name="atq")
                            for kt in range(NKT):
                                nc.tensor.matmul(
                                    atq[:, 0 : HD + 1],
                                    prb[:, kt, qt * 128 : (qt + 1) * 128],
                                    vaug[:, b, kt, h, :],
                                    start=(kt == 0),
                                    stop=(kt == NKT - 1),
                                    skip_group_check=True,
                                )
                            # normalize on eviction: row sums are per-PARTITION
                            # scalars in the qi-natural layout
                            rc = nrm.tile([128, 1], f32, tag="rc", name="rc")
                            nc.vector.reciprocal(rc[:], atq[:, HD : HD + 1])
                            nc.vector.tensor_scalar_mul(
                                a_nat[:, qt, :], atq[:, 0:HD], rc[:]
                            )

                        return thunk

                    # qt order alternates the two at_ps banks so bank N's next
                    # group starts only after its previous group's eviction
                    return [grp(qt) for qt in (0, 4, 1, 5, 2, 6, 3, 7)]

                def finish_block(b, qb):
                    # rebuild attnT [2h*64d, qi] for the out-projection
                    for h in range(HPC):
                        hs = slice(h * HD, (h + 1) * HD)
                        a_nat = a_nats[(b, qb)][h]
                        for half in range(2):
                            tr = mix_tile()[0:HD, 0:256].bitcast(f16)
                            for qt4 in range(4):
                                qt = half * 4 + qt4
                                nc.tensor.transpose(
                                    tr[:, qt4 * 128 : (qt4 + 1) * 128],
                                    a_nat[:, qt, :],
                                    id16[:],
                                )
                            nc.vector.tensor_copy(
                                attnT[b][qb][hs, half * 512 : (half + 1) * 512],
                                tr[:],
                            )
                    pending.extend(make_outproj(b, qb, tail=(b, qb) == (1, 1)))
                    if (b, qb) == (0, 1):
                        pending.append(lambda: phase3(0))

                def emit_sc_u(u, kt):
                    ub, uqb, uh = u
                    uq0 = ub * S + uqb * QB
                    uhs = slice(uh * HD, (uh + 1) * HD)
                    k0 = ub * S + kt * 128
                    sc = sc_ps.tile([128, QB], f32, tag="sc", name="sc")
                    for half in range(2):
                        nc.tensor.matmul(
                            sc[:, half * 512 : (half + 1) * 512],
                            kT[uhs, k0 : k0 + 128],
                            qT[uhs, uq0 + half * 512 : uq0 + (half + 1) * 512],
                            start=True,
                            stop=True,
                            skip_group_check=True,
                        )
                    return sc

                # scores run one kt ahead of exp -- across unit boundaries
                # too -- so PE bursts (thunks, unit-end drains) never starve
                # the ACT pacer
                avq: list = []  # previous unit's av-group thunks
                prev_unit = None
                scs = None
                for i, u in enumerate(units):
                    b, qb, h = u
                    prb = probs_pool.tile(
                        [128, NKT, QB], f16, tag="prb", name="prb"
                    )
                    if scs is None:
                        scs = emit_sc_u(u, 0)
                    for kt in range(NKT):
                        sc = scs
                        scs = emit_sc_u(u, kt + 1) if kt + 1 < NKT else None
                        nc.scalar.activation(prb[:, kt, :], sc[:], Exp)
                        if kt % 2 == 1 and avq:
                            avq.pop(0)()
                        emit_some(1)
                    if i + 1 < len(units):
                        scs = emit_sc_u(units[i + 1], 0)
                    for t in avq:
                        t()
                    avq = av_groups(b, qb, h, prb)
                    if prev_unit is not None and prev_unit[2] == HPC - 1:
                        finish_block(prev_unit[0], prev_unit[1])
                    prev_unit = u
                for t in avq:
                    t()
                finish_block(prev_unit[0], prev_unit[1])
                emit_some(len(pending))
                phase3(1)
                ph2c.__exit__(None, None, None)
                ph2b.__exit__(None, None, None)
                ph2.__exit__(None, None, None)

    nc.compile()
    return nc


def _host_inputs(query, Wq, bq, Wk, bk, Wv, bv, Wo, bo):
    """Per-core input maps."""
    scaling = HD ** (-0.5)

    invf = 1.0 / (
        10000.0 ** (np.arange(0, HD, 2, dtype=np.float32) / np.float32(HD))
    )
    t = np.arange(S, dtype=np.float32)
    fr = np.outer(t, invf).astype(np.float32)  # [S, 32]
    emb = np.concatenate([fr, fr], axis=1)  # [S, HD]
    cosT = np.cos(emb).T.astype(np.float32)  # [HD, S]
    sinT = np.sin(emb).T.astype(np.float32)
    sign = np.where(np.arange(HD) < HD // 2, -1.0, 1.0).astype(np.float32)[:, None]
    cos_t = np.ascontiguousarray(np.tile(cosT, (HPC, 1))).astype(np.float16)
    sin_t = np.ascontiguousarray(np.tile(sinT * sign, (HPC, 1))).astype(np.float16)

    query16 = np.ascontiguousarray(np.asarray(query, dtype=np.float16))
    in_maps = []
    for c in range(NCORES):
        sl = slice(c * EL, (c + 1) * EL)
        in_maps.append(
            {
                "query": query16,
                "q_w": np.ascontiguousarray((Wq[sl, :] * scaling).T).astype(
                    np.float16
                ),
                "k_w": np.ascontiguousarray(Wk[sl, :].T).astype(np.float16),
                "v_w": np.ascontiguousarray(Wv[sl, :].T).astype(np.float16),
                "o_w": np.ascontiguousarray(Wo[:, sl].T).astype(np.float16),
                "bq_s": np.ascontiguousarray(
                    (bq[sl] * scaling).reshape(EL, 1), dtype=np.float32
                ),
                "bk_s": np.ascontiguousarray(bk[sl].reshape(EL, 1), dtype=np.float32),
                "cos_t": cos_t,
                "sin_t": sin_t,
            }
        )
    return in_maps


def kernel(query, Wq, bq, Wk, bk, Wv, bv, Wo, bo):
    global LAST_RESULT
    from concourse.bass_utils import run_bass_kernel_spmd

    if "nc" not in _CACHE:
        _CACHE["nc"] = _build_program()
    nc = _CACHE["nc"]

    in_maps = _host_inputs(
        np.asarray(query),
        np.asarray(Wq),
        np.asarray(bq),
        np.asarray(Wk),
        np.asarray(bk),
        np.asarray(Wv),
        np.asarray(bv),
        np.asarray(Wo),
        np.asarray(bo),
    )
    res = run_bass_kernel_spmd(nc, in_maps, core_ids=list(range(NCORES)))
    LAST_RESULT = res
    # shards: [B, SHARD, E] fp16 per core; core c covers rows
    # c*SHARD:(c+1)*SHARD of each batch's [S, E] partial-sum output.
    shards = np.stack(
        [res.results[c]["out"].astype(np.float32) for c in range(NCORES)]
    )  # [C, B, SHARD, E]
    full = shards.transpose(1, 0, 2, 3).reshape(B, S, E)  # [B, S, E]
    # v's bias is exact as a constant output shift (softmax sums to 1):
    # out += bv @ Wo.T + bo, applied host-side after unsharding.
    bo_eff = (
        np.asarray(bo, dtype=np.float32)
        + np.asarray(bv, dtype=np.float32) @ np.asarray(Wo, dtype=np.float32).T
    )
    out = full.transpose(1, 0, 2) + bo_eff
    return np.ascontiguousarray(out)

